# revision 2
# baseline (speedup 1.0000x reference)
"""Cluster-based contrastive loss on 8 Trainium2 NeuronCores — v2.

Layout: cluster c = 8*s + pid (slot s in [0,7), core pid) lives at table
columns [256c, 256c+256).  Real clusters: slots 0-5 on all cores (48) plus
slot 6 on cores 0-1 (clusters 48, 49).  Padding [12800, 14336) never swept.

Per core:
  - 7x kth_largest thresholds (gpsimd), per-slot top-128 extraction via
    max8 + one-hot [128,128,8] + 8 PSUM-accumulated PE matmuls,
  - 4-phase (2+2+2+1 slots) dma_gather / normalize / PE-transpose /
    AllGather pipeline with single strided reload DMAs,
  - own-block pre-pass on local columns gives pos/own sums with static
    APs while the collectives are in flight,
  - sweep: 13 balanced row-blocks (12 own + 1 dyn helper) x 12800 cols,
    exp on ACT (single act table) with accum_out row sums,
  - ln(neg)-ln(pos), weighted scalar out; host sums the 8 partials.
"""

import sys

sys.path.insert(0, "/opt/trn_rl_repo")

import numpy as np

import concourse.bacc as bacc
import concourse.bass as bass
import concourse.mybir as mybir
from concourse import tile
from concourse.bass_utils import run_bass_kernel_spmd

F32 = mybir.dt.float32
BF16 = mybir.dt.bfloat16
I16 = mybir.dt.int16
AF = mybir.ActivationFunctionType
ALU = mybir.AluOpType

B = 16384
D = 128
C = 50
K = 128
TEMP = 0.5
N_CORES = 8
SLOTS = 7
T8 = 8
TBL = 14336            # 56 slots * 256 (padded); real cols [0, 12800)
NROW = 13              # swept row-blocks per core (12 own + 1 helper)
CHUNKS = [(0, 2048), (2048, 2048), (4096, 2048),
          (6144, 2048), (8192, 2048), (10240, 2048), (12288, 512)]
PHASES = [(0, 2), (2, 4), (4, 6), (6, 7)]
QUANTILE = 1.0 - 127.5 / (B - 1)
# packed const columns: iota1 | jiota | lexcl | ident | t8 | wfin
CIOTA1, CJIOTA, CLEXCL, CIDENT = 0, 128, 256, 384
CT8, CWFIN = 512, 520
CPACKW = 533

_CACHE = {}


def _host_constants():
    pack = np.zeros((128, CPACKW), dtype=np.float32)
    pack[:, CIOTA1:CIOTA1 + 128] = (
        np.arange(128)[:, None] * 128 + np.arange(128)[None, :] + 1
    )
    pack[:, CJIOTA:CJIOTA + 128] = np.arange(128, dtype=np.float32)[None, :]
    pack[:, CLEXCL:CLEXCL + 128] = (
        np.arange(128)[:, None] < np.arange(128)[None, :]
    )
    pack[:, CIDENT:CIDENT + 128] = np.eye(128)
    pack[:, CT8:CT8 + 8] = np.arange(8, dtype=np.float32)[None, :]
    rep16 = (np.arange(128)[None, :] % 16 == np.arange(16)[:, None]).astype(
        np.float32
    )
    return pack, rep16


def _build_program(reps=1):
    nc = bacc.Bacc(
        "TRN2", target_bir_lowering=False, debug=False, num_devices=N_CORES
    )

    probT = nc.dram_tensor("probT", [SLOTS, B], F32, kind="ExternalInput")
    z_i = nc.dram_tensor("z_i", [B, D], F32, kind="ExternalInput")
    z_j = nc.dram_tensor("z_j", [B, D], F32, kind="ExternalInput")
    cpack = nc.dram_tensor("cpack", [128, CPACKW], F32, kind="ExternalInput")
    rep16 = nc.dram_tensor("rep16", [16, 128], F32, kind="ExternalInput")
    out = nc.dram_tensor("partial", [1, 1], F32, kind="ExternalOutput")

    with tile.TileContext(nc) as tc:
        for rep in range(reps):
            _emit(nc, tc, probT, z_i, z_j, cpack, rep16, out)

    nc.compile()
    return nc


def _emit(nc, tc, probT, z_i, z_j, cpack, rep16, out):
    from contextlib import ExitStack

    ctx = ExitStack()
    with ctx:
        const = ctx.enter_context(tc.tile_pool(name="const", bufs=1))
        main = ctx.enter_context(tc.tile_pool(name="main", bufs=1))
        scr = ctx.enter_context(tc.tile_pool(name="scr", bufs=2))
        dram = ctx.enter_context(tc.tile_pool(name="dram", bufs=1, space="DRAM"))
        setup_ctx = ExitStack()
        psA = setup_ctx.enter_context(tc.tile_pool(name="psA", bufs=2, space="PSUM"))
        psB = setup_ctx.enter_context(tc.tile_pool(name="psB", bufs=1, space="PSUM"))

        pid = nc.partition_id()

        # ---- inputs: prob slots first (kth starts early), then consts --
        prob_sb = main.tile([128, SLOTS, 128], F32, tag="prob")
        pk = const.tile([128, CPACKW], F32, tag="pack")
        rep16_sb = const.tile([16, 128], F32, tag="rep16")
        for s in range(2):
            nc.sync.dma_start(
                prob_sb[:, s : s + 1, :],
                probT.ap()[s : s + 1, :].rearrange("c (p f) -> p c f", p=128),
            )
        nc.sync.dma_start(pk[:], cpack.ap())
        nc.sync.dma_start(rep16_sb[:], rep16.ap())
        for s in range(2, SLOTS):
            nc.sync.dma_start(
                prob_sb[:, s : s + 1, :],
                probT.ap()[s : s + 1, :].rearrange("c (p f) -> p c f", p=128),
            )
        iota1_sb = pk[:, CIOTA1:CIOTA1 + 128]
        jiota_sb = pk[:, CJIOTA:CJIOTA + 128]
        lexcl_sb = pk[:, CLEXCL:CLEXCL + 128]
        ident_sb = pk[:, CIDENT:CIDENT + 128]
        t8_sb = pk[:, CT8:CT8 + 8]
        wfin_sb = pk[:, CWFIN:CWFIN + NROW]
        ones_p = const.tile([128, 1], F32, tag="ones_p")
        ones_r = const.tile([1, 128], F32, tag="ones_r")
        nc.vector.memset(ones_p[:], 1.0)
        nc.vector.memset(ones_r[:], 1.0)

        # ---- stage A: thresholds ---------------------------------------
        taus = main.tile([1, 2 * SLOTS], F32, tag="taus")
        for s in range(SLOTS):
            nc.gpsimd.kth_largest(
                taus[0:1, 2 * s : 2 * s + 2],
                prob_sb[:, s, :],
                n_per_lane=128,
                k=K + 2,
                quantile=QUANTILE,
            )

        # ---- stage B: per-slot top-128 index extraction ----------------
        allidx = main.tile([1, SLOTS * 128], F32, tag="allidx")
        for s in range(SLOTS):
            taub = psB.tile([128, 1], F32, tag="taub")
            nc.tensor.matmul(taub[:], ones_r[:], taus[0:1, 2 * s + 1 : 2 * s + 2])
            ge = scr.tile([128, 128], F32, tag="ge")
            nc.vector.tensor_tensor(
                ge[:], prob_sb[:, s, :], taub[:].to_broadcast([128, 128]),
                op=ALU.is_gt,
            )
            msk = scr.tile([128, 128], F32, tag="msk")
            nc.vector.scalar_tensor_tensor(
                msk[:], ge[:], 0.0, iota1_sb, op0=ALU.add, op1=ALU.mult
            )
            nc.vector.tensor_scalar_add(msk[:], msk[:], -1.0)
            cand = scr.tile([128, T8], F32, tag="cand")
            nc.vector.max(cand[:], msk[:])
            valid = scr.tile([128, T8], F32, tag="valid")
            nc.vector.tensor_scalar(valid[:], cand[:], -0.5, None, op0=ALU.is_gt)
            rowcnt = scr.tile([128, 1], F32, tag="rowcnt")
            nc.vector.tensor_reduce(
                rowcnt[:], valid[:], axis=mybir.AxisListType.X, op=ALU.add
            )
            rowoff = psB.tile([128, 1], F32, tag="rowoff")
            nc.tensor.matmul(rowoff[:], lexcl_sb, rowcnt[:])
            rank = scr.tile([128, T8], F32, tag="rank")
            nc.vector.tensor_tensor(
                rank[:], rowoff[:].to_broadcast([128, T8]), t8_sb, op=ALU.add
            )
            nc.vector.tensor_scalar_add(rank[:], rank[:], -999.0)
            nc.vector.tensor_tensor(rank[:], rank[:], valid[:], op=ALU.mult)
            nc.vector.tensor_scalar_add(rank[:], rank[:], 999.0)
            ev3 = scr.tile([128, 128, T8], F32, tag="ev3")
            nc.vector.tensor_tensor(
                ev3[:],
                rank[:].rearrange("p (o t) -> p o t", o=1).to_broadcast([128, 128, T8]),
                jiota_sb.rearrange("p (j o) -> p j o", o=1).to_broadcast([128, 128, T8]),
                op=ALU.is_equal,
            )
            acc = psB.tile([128, 1], F32, tag="acc")
            for t in range(T8):
                nc.tensor.matmul(
                    acc[:], ev3[:, :, t], cand[:, t : t + 1],
                    start=(t == 0), stop=(t == T8 - 1),
                )
            idxv = scr.tile([128, 1], F32, tag="idxv")
            nc.vector.tensor_copy(idxv[:], acc[:])
            tr = psB.tile([1, 128], F32, tag="tr")
            nc.tensor.matmul(tr[:], idxv[:], ident_sb)
            nc.scalar.copy(allidx[0:1, 128 * s : 128 * (s + 1)], tr[:])

        # ---- stages C-F, per phase: idx wrap, gather, normalize+T, AG --
        locT = main.tile([128, SLOTS, 256], BF16, tag="locT")
        sqs = main.tile([128, 2 * SLOTS], F32, tag="sqs")
        idxs_i16 = main.tile([128, SLOTS * 8], I16, tag="idxs")
        gi = main.tile([128, SLOTS, 128], F32, tag="gi")
        gj = main.tile([128, SLOTS, 128], F32, tag="gj")
        flatT = main.tile([128, TBL], BF16, tag="flatT")
        flat4 = flatT[:].rearrange("p (s j c) -> p s j c", s=SLOTS, j=N_CORES)
        reloads = []
        for s0, s1 in PHASES:
            ns = s1 - s0
            idxd = dram.tile([1, ns * 128], F32)
            nc.sync.dma_start(idxd[:], allidx[0:1, 128 * s0 : 128 * s1])
            wrapped = scr.tile([16, ns * 8], F32, tag=f"wrap{s0}")
            nc.sync.dma_start(
                wrapped[:], idxd[:].rearrange("p (s m) -> (p m) s", m=16)
            )
            widx = psB.tile([128, 16], F32, tag="widx")
            nc.tensor.matmul(widx[:, : ns * 8], rep16_sb[:], wrapped[:])
            nc.vector.tensor_copy(
                idxs_i16[:, 8 * s0 : 8 * s1], widx[:, : ns * 8]
            )
            for g_sb, z in ((gi, z_i), (gj, z_j)):
                nc.gpsimd.dma_gather(
                    g_sb[:, s0:s1, :],
                    z.ap(),
                    idxs_i16[:, 8 * s0 : 8 * s1],
                    num_idxs=ns * 128,
                    num_idxs_reg=ns * 128,
                    elem_size=D,
                )
            for s in range(s0, s1):
                for h, g_sb in ((0, gi), (1, gj)):
                    src = g_sb[:, s, :]
                    trash = scr.tile([128, 128], F32, tag="trash")
                    nc.scalar.activation(
                        trash[:], src, AF.Square,
                        accum_out=sqs[:, 2 * s + h : 2 * s + h + 1],
                    )
                nrm = scr.tile([128, 2], F32, tag="nrm")
                nc.scalar.activation(nrm[:], sqs[:, 2 * s : 2 * s + 2], AF.Sqrt)
                rn = scr.tile([128, 2], F32, tag="rn")
                nc.vector.reciprocal(rn[:], nrm[:])
                for h, g_sb in ((0, gi), (1, gj)):
                    diag = scr.tile([128, 128], F32, tag="diag")
                    nc.vector.tensor_tensor(
                        diag[:], ident_sb,
                        rn[:, h : h + 1].to_broadcast([128, 128]), op=ALU.mult,
                    )
                    tp = psA.tile([128, 128], F32, tag="tp")
                    nc.tensor.matmul(tp[:], g_sb[:, s, :], diag[:])
                    nc.vector.tensor_copy(
                        locT[:, s, 128 * h : 128 * h + 128], tp[:]
                    )
            # AllGather this phase; reloads queued after the last agin
            agin = dram.tile([128, ns * 256], BF16)
            agout = dram.tile([N_CORES * 128, ns * 256], BF16, addr_space="Shared")
            nc.sync.dma_start(agin[:], locT[:, s0:s1, :])
            nc.gpsimd.collective_compute(
                "AllGather",
                ALU.bypass,
                replica_groups=[list(range(N_CORES))],
                ins=[agin.opt()],
                outs=[agout.opt()],
            )
            reloads.append((s0, s1, ns, agout))
        for s0, s1, ns, agout in reloads:
            for s in range(s0, s1):
                nc.sync.dma_start(
                    flat4[:, s : s + 1, :, :],
                    agout[:, 256 * (s - s0) : 256 * (s - s0 + 1)].rearrange(
                        "(j p) c -> p j c", p=128
                    ),
                )

        # ---- pre-pass: own rows x local columns (pos/own sums) ---------
        setup_ctx.close()
        pre_ctx = ExitStack()
        ps_pre = pre_ctx.enter_context(
            tc.tile_pool(name="ps_pre", bufs=2, space="PSUM")
        )
        escr = ctx.enter_context(tc.tile_pool(name="escr", bufs=4))
        partials = main.tile([128, NROW, len(CHUNKS)], F32, tag="partials")
        pos_t = main.tile([128, NROW], F32, tag="pos_t")
        own_t = main.tile([128, NROW], F32, tag="own_t")
        loc2 = locT[:].rearrange("p s c -> p (s c)")
        LOCW = SLOTS * 256  # 1792
        for jj in range(12):
            s, h = jj >> 1, jj & 1
            lhsT = locT[:, s, 128 * h : 128 * h + 128]
            simp = ps_pre.tile([128, LOCW], F32, tag="simp")
            for q in range(0, LOCW, 512):
                w = min(512, LOCW - q)
                nc.tensor.matmul(simp[:, q : q + w], lhsT, loc2[:, q : q + w])
            e_pre = escr.tile([128, LOCW], BF16, tag="epre")
            nc.scalar.activation(e_pre[:], simp[:], AF.Exp, scale=1.0 / TEMP)
            nc.vector.tensor_reduce(
                pos_t[:, jj : jj + 1], e_pre[:, 256 * s : 256 * s + 128],
                axis=mybir.AxisListType.X, op=ALU.add,
            )
            nc.vector.tensor_reduce(
                own_t[:, jj : jj + 1], e_pre[:, 256 * s : 256 * s + 256],
                axis=mybir.AxisListType.X, op=ALU.add,
            )
        pre_ctx.close()

        # ---- stage G: sweep --------------------------------------------
        sweep_ctx = ExitStack()
        ps_sim = sweep_ctx.enter_context(
            tc.tile_pool(name="ps_sim", bufs=2, space="PSUM")
        )

        helper_idx = pid & 3
        lhs_help = main.tile([128, 128], BF16, tag="lhs_help")
        nc.sync.dma_start(
            lhs_help[:], flatT[:, bass.ds(12288 + 128 * helper_idx, 128)]
        )

        def lhsT_of(jj):
            if jj < 12:
                s, h = jj >> 1, jj & 1
                return locT[:, s, 128 * h : 128 * h + 128]
            return lhs_help[:]

        def sweep_chunk(jj, k):
            c0, w = CHUNKS[k]
            sim = ps_sim.tile([128, 2048], F32, tag="sim")
            for q in range(0, w, 512):
                nc.tensor.matmul(
                    sim[:, q : q + 512],
                    lhsT_of(jj),
                    flatT[:, c0 + q : c0 + q + 512],
                )
            e_sb = escr.tile([128, 2048], BF16, tag="e")
            nc.scalar.activation(
                e_sb[:, :w],
                sim[:, :w],
                AF.Exp,
                scale=1.0 / TEMP,
                accum_out=partials[:, jj, k : k + 1],
            )
            return e_sb

        def own_reduces(jj, e_sb, off):
            nc.vector.tensor_reduce(
                pos_t[:, jj : jj + 1], e_sb[:, bass.ds(off, 128)],
                axis=mybir.AxisListType.X, op=ALU.add,
            )
            nc.vector.tensor_reduce(
                own_t[:, jj : jj + 1], e_sb[:, bass.ds(off, 256)],
                axis=mybir.AxisListType.X, op=ALU.add,
            )

        # chunk-major passes following the AllGather phases
        for s0, s1 in PHASES:
            for k in range(s0, s1):
                for jj in range(12):
                    sweep_chunk(jj, k)
        # helper row last (needs the final reload)
        for k in range(len(CHUNKS)):
            e_sb = sweep_chunk(12, k)
            if k == len(CHUNKS) - 1:
                own_reduces(12, e_sb, 256 * (helper_idx >> 1))

        # ---- stage H: logs + weighted scalar ---------------------------
        total = main.tile([128, NROW], F32, tag="total")
        for jj in range(NROW):
            nc.vector.tensor_reduce(
                total[:, jj : jj + 1], partials[:, jj, :],
                axis=mybir.AxisListType.X, op=ALU.add,
            )
        neg = main.tile([128, NROW], F32, tag="neg")
        nc.vector.tensor_sub(neg[:], total[:], own_t[:])
        lnn = main.tile([128, NROW], F32, tag="lnn")
        lnp = main.tile([128, NROW], F32, tag="lnp")
        nc.scalar.activation(lnn[:], neg[:], AF.Ln)
        nc.scalar.activation(lnp[:], pos_t[:], AF.Ln)
        lsub = main.tile([128, NROW], F32, tag="lsub")
        nc.vector.tensor_sub(lsub[:], lnn[:], lnp[:])
        nc.vector.tensor_tensor(lsub[:], lsub[:], wfin_sb, op=ALU.mult)
        rowagg = main.tile([128, 1], F32, tag="rowagg")
        nc.vector.tensor_reduce(
            rowagg[:], lsub[:], axis=mybir.AxisListType.X, op=ALU.add
        )
        sweep_ctx.close()
        tail_ctx = ExitStack()
        ps_t = tail_ctx.enter_context(tc.tile_pool(name="ps_t", bufs=1, space="PSUM"))
        fin = ps_t.tile([1, 1], F32, tag="fin")
        nc.tensor.matmul(fin[:], ones_p[:], rowagg[:])
        out_sb = main.tile([1, 1], F32, tag="out_sb")
        nc.vector.tensor_scalar_mul(out_sb[:], fin[:], 1.0 / (2 * K * C))
        nc.sync.dma_start(out.ap(), out_sb[:])
        tail_ctx.close()


def _per_core_inputs(prob, z_i, z_j):
    pack, rep16 = _host_constants()
    maps = []
    for k in range(N_CORES):
        cols = []
        for s in range(SLOTS):
            c = 8 * s + k
            cols.append(c if c < C else k)
        pT = np.ascontiguousarray(prob[:, cols].T)
        pk = pack.copy()
        pk[:, CWFIN:CWFIN + NROW] = 1.0
        if k >= 4:
            pk[:, CWFIN + 12] = 0.0
        m = {"probT": pT, "z_i": z_i, "z_j": z_j, "cpack": pk, "rep16": rep16}
        maps.append(m)
    return maps


def kernel(prob, z_i, z_j):
    if "nc" not in _CACHE:
        _CACHE["nc"] = _build_program()
    nc = _CACHE["nc"]
    in_maps = _per_core_inputs(
        np.asarray(prob, dtype=np.float32),
        np.ascontiguousarray(z_i, dtype=np.float32),
        np.ascontiguousarray(z_j, dtype=np.float32),
    )
    res = run_bass_kernel_spmd(nc, in_maps, list(range(N_CORES)))
    total = np.float32(0.0)
    for r in res.results:
        total += r["partial"][0, 0]
    return np.asarray(total, dtype=np.float32)
